# revision 10
# baseline (speedup 1.0000x reference)
"""DecoderTreeLSTMCell Trainium2 Bass kernel (v4).

Strategy: data-parallel over nodes on 8 cores (4096 nodes/core). Host-side,
each core's nodes are sorted by `pos` (10 blocks); within each pos block
[mask=0 | mask=1], each side sub-ordered [depth!=1,2 | d==1 | d==2] with
padded compile-time capacities. All device I/O is bf16 (rel-err gate is 2e-2;
bf16 lands ~6e-3), which halves HBM traffic vs fp32 AND avoids the fp32r
<256-col 4x PE streaming penalty.

AIN is laid out in 5 group blocks (2 positions each): [h | c | e] per group,
DMA'd in 3 slabs (~0.9MB each) so compute starts ~1.5us into a pass while
later slabs stream. OUT has global regions [h_new | c_new | c_red_mask1]
written by 3 DMAs as each region completes.

The depth-1/2 "extra input" adds are folded into the matmuls: gate matmuls
accumulate W^T @ e over the depth-1,2 column sub-ranges directly in PSUM
(same stationary weights, start=False) - exact same math as adding e to
h_cat, minus 20 vector ops. The i/o/u-gate matmuls of a 2-pos group share
PSUM banks ([i_p0|i_p1]...) so each sigmoid/tanh is ONE ACT instruction per
group (ACT's ~190ns/instr bubble is the #1 cost). Forget-gate f is per-pos
(bias b_f[pos] applied free by ACT). Elementwise work runs on DVE in bf16
over 1024-col spans of body-wide tiles.

The timing loop fuses 4 harness iterations per For_i trip (8 passes/trip):
the For_i back-edge costs an all-engine barrier + pipeline refill, a harness
artifact that would otherwise be charged to every 2nd pass. Total pass count
is exactly reps*bodies (remainder passes are emitted outside the loop).
mask=1 rows of h equal h_prev exactly and are filled host-side during
unshard (data routing only - all arithmetic happens on device).
"""
import numpy as np
import ml_dtypes

import concourse.bacc as bacc
import concourse.mybir as mybir
from concourse.tile import TileContext
from concourse.bass_utils import run_bass_kernel_spmd

N = 32768
H = 128
N_POS = 10
NC = 8
SH = N // NC  # nodes per core

F32 = mybir.dt.float32
BF16 = mybir.dt.bfloat16
BF = ml_dtypes.bfloat16
Sig = mybir.ActivationFunctionType.Sigmoid
Tanh = mybir.ActivationFunctionType.Tanh

FUSE = 8          # harness iterations fused per For_i trip
IN_SLABS = ((0, 2), (2, 4), (4, 5))   # group ranges per input DMA

# module-level stash for test harness introspection
LAST = {}


def _roundup(x, m):
    return ((x + m - 1) // m) * m


def _plan(pos, depth, mask):
    """Per-core slot layout: pos-major blocks [m0:k0,k1,k2 | m1:k0,k1,k2].

    AIN: 5 group blocks (2 pos each) of [h | c | e]. OUT: global regions
    [h_new (M0t) | c_new (M0t) | c_red_mask1 (M1t)].
    """
    dcl = np.where(depth == 1, 1, np.where(depth == 2, 2, 0))
    idx = {}
    counts = np.zeros((NC, N_POS, 2, 3), np.int64)
    # deal each (pos, mask, dclass) bucket round-robin across cores so
    # per-core counts are equal +-1 -> capacities carry almost no padding
    for p in range(N_POS):
        for m in range(2):
            for k in range(3):
                gg = np.nonzero((pos == p) & (mask == m) & (dcl == k))[0]
                for c in range(NC):
                    ii = gg[c::NC]
                    idx[(c, p, m, k)] = ii
                    counts[c, p, m, k] = len(ii)

    caps = np.zeros((N_POS, 2, 3), np.int64)
    for p in range(N_POS):
        for m in range(2):
            for k in range(3):
                caps[p, m, k] = _roundup(int(counts[:, p, m, k].max()), 4)

    M0 = caps[:, 0, :].sum(axis=1)       # [N_POS]
    M1 = caps[:, 1, :].sum(axis=1)
    C = M0 + M1
    P_off = np.concatenate([[0], np.cumsum(C)])       # slot offset per pos
    M0_off = np.concatenate([[0], np.cumsum(M0)])
    M1_off = np.concatenate([[0], np.cumsum(M1)])
    L = int(P_off[-1])
    M0t, M1t = int(M0.sum()), int(M1.sum())
    assert C.max() <= 512, f"pos block too wide for a PSUM bank: {C.max()}"

    slot_idx = np.full((NC, L), -1, np.int64)
    for c in range(NC):
        for p in range(N_POS):
            for m in range(2):
                o = int(P_off[p] + (0 if m == 0 else M0[p]))
                for k in range(3):
                    ii = idx[(c, p, m, k)]
                    slot_idx[c, o:o + len(ii)] = ii
                    o += int(caps[p, m, k])

    groups = []
    ain_slot, ain_kind = [], []
    a = 0
    for g in range(N_POS // 2):
        p0, p1 = 2 * g, 2 * g + 1
        Cg = int(C[p0] + C[p1])
        M0g = int(M0[p0] + M0[p1])
        assert M0g <= 512
        poss = []
        hrel = 0
        eoff = 2 * Cg
        for p in (p0, p1):
            w0 = int(caps[p, 0, 1] + caps[p, 0, 2])
            w1 = int(caps[p, 1, 1] + caps[p, 1, 2])
            poss.append((p, int(C[p]), int(M0[p]), int(M1[p]), hrel,
                         eoff, w0, eoff + w0, w1,
                         int(M0_off[p]), int(M1_off[p]), int(P_off[p])))
            eoff += w0 + w1
            hrel += int(C[p])
        Ag = eoff
        groups.append(dict(a0=a, Ag=Ag, Cg=Cg, M0g=M0g,
                           gm0=int(M0_off[p0]), poss=poss))
        # AIN columns for this group: h | c | e
        for p in (p0, p1):
            ain_slot.extend(range(int(P_off[p]), int(P_off[p] + C[p])))
            ain_kind.extend([0] * int(C[p]))
        for p in (p0, p1):
            ain_slot.extend(range(int(P_off[p]), int(P_off[p] + C[p])))
            ain_kind.extend([1] * int(C[p]))
        for p in (p0, p1):
            w0 = int(caps[p, 0, 1] + caps[p, 0, 2])
            ain_slot.extend(range(int(P_off[p] + M0[p] - w0),
                                  int(P_off[p] + M0[p])))
            ain_kind.extend([2] * w0)
            w1 = int(caps[p, 1, 1] + caps[p, 1, 2])
            ain_slot.extend(range(int(P_off[p] + C[p] - w1),
                                  int(P_off[p] + C[p])))
            ain_kind.extend([2] * w1)
        a += Ag
    Lin = a

    out_slot, out_kind = [], []
    for p in range(N_POS):
        out_slot.extend(range(int(P_off[p]), int(P_off[p] + M0[p])))
        out_kind.extend([0] * int(M0[p]))
    for p in range(N_POS):
        out_slot.extend(range(int(P_off[p]), int(P_off[p] + M0[p])))
        out_kind.extend([1] * int(M0[p]))
    for p in range(N_POS):
        out_slot.extend(range(int(P_off[p] + M0[p]), int(P_off[p] + C[p])))
        out_kind.extend([2] * int(M1[p]))
    Lout = 2 * M0t + M1t

    return dict(groups=groups, L=L, Lin=Lin, Lout=Lout, M0t=M0t, M1t=M1t,
                slot_idx=slot_idx,
                ain_slot=np.array(ain_slot), ain_kind=np.array(ain_kind),
                out_slot=np.array(out_slot), out_kind=np.array(out_kind))


def _build(plan, reps=1, bodies=1):
    groups = plan["groups"]
    Lin, Lout = plan["Lin"], plan["Lout"]
    M0t, M1t = plan["M0t"], plan["M1t"]
    nc = bacc.Bacc("TRN2", target_bir_lowering=False)
    AIN = nc.dram_tensor("AIN", [H, Lin], BF16, kind="ExternalInput")
    W = nc.dram_tensor("W", [H, N_POS * 4 * H], BF16, kind="ExternalInput")
    BIAS = nc.dram_tensor("BIAS", [H, 13], F32, kind="ExternalInput")
    OUT = nc.dram_tensor("OUT", [H, Lout], BF16, kind="ExternalOutput")

    with TileContext(nc) as tc:
        with (
            tc.tile_pool(name="const", bufs=1) as cpool,
            tc.tile_pool(name="io", bufs=2) as io,
            tc.tile_pool(name="gates", bufs=2) as gates,
            tc.tile_pool(name="wk", bufs=2) as wk,
            tc.tile_pool(name="ps_u", bufs=2, space="PSUM") as ps_u,
            tc.tile_pool(name="ps_iot", bufs=2, space="PSUM") as ps_iot,
        ):
            bias_sb = cpool.tile([H, 13], F32, tag="bias")
            nc.sync.dma_start(out=bias_sb[:, :], in_=BIAS[:, :])
            w_tiles = {}

            def w_load(p):
                if p not in w_tiles:
                    t = cpool.tile([H, 4 * H], BF16, tag=f"w{p}")
                    nc.sync.dma_start(
                        out=t[:, :], in_=W[:, p * 4 * H:(p + 1) * 4 * H])
                    w_tiles[p] = t
                return w_tiles[p]

            def body(_iv=None):
                ain = io.tile([H, Lin], BF16, tag="ain")
                for (glo, ghi) in IN_SLABS:
                    s0 = groups[glo]["a0"]
                    s1 = (groups[ghi]["a0"] if ghi < len(groups) else Lin)
                    nc.sync.dma_start(out=ain[:, s0:s1], in_=AIN[:, s0:s1])
                out = io.tile([H, Lout], BF16, tag="out")
                f_all = gates.tile([H, plan["L"]], BF16, tag="f")
                si_all = gates.tile([H, M0t], BF16, tag="si")
                so_all = gates.tile([H, M0t], BF16, tag="so")
                tu_all = gates.tile([H, M0t], BF16, tag="tu")
                cr0 = wk.tile([H, M0t], BF16, tag="cr0")
                prod = wk.tile([H, M0t], BF16, tag="pr")
                th = wk.tile([H, M0t], BF16, tag="th")

                for blk in groups:
                    a0, Cg, M0g, gm0 = (blk["a0"], blk["Cg"], blk["M0g"],
                                        blk["gm0"])
                    # u matmuls + forget gate, per pos (e adds folded into
                    # PSUM accumulation; u split at the mask0/1 boundary so
                    # each e block is a suffix of its segment)
                    for (p, C, M0, M1, hrel, e0, w0, e1, w1, m0o,
                         m1o, po) in blk["poss"]:
                        w_sb = w_load(p)
                        ho = a0 + hrel
                        pu = ps_u.tile([H, C], F32, tag="u")
                        nc.tensor.matmul(pu[:, 0:M0], w_sb[:, 0:H],
                                         ain[:, ho:ho + M0],
                                         start=True, stop=(w0 == 0))
                        if w0:
                            nc.tensor.matmul(pu[:, M0 - w0:M0], w_sb[:, 0:H],
                                             ain[:, a0 + e0:a0 + e0 + w0],
                                             start=False, stop=True)
                        nc.tensor.matmul(pu[:, M0:C], w_sb[:, 0:H],
                                         ain[:, ho + M0:ho + C],
                                         start=True, stop=(w1 == 0))
                        if w1:
                            nc.tensor.matmul(pu[:, C - w1:C], w_sb[:, 0:H],
                                             ain[:, a0 + e1:a0 + e1 + w1],
                                             start=False, stop=True)
                        nc.scalar.activation(f_all[:, po:po + C],
                                             pu[:, :], Sig,
                                             bias=bias_sb[:, p:p + 1])

                    # i/o/u gates: both positions share PSUM banks
                    pio = ps_iot.tile([H, 1536], F32, tag="iot")
                    for j in range(3):
                        boff = 512 * j
                        for (p, C, M0, M1, hrel, e0, w0, e1, w1, m0o,
                             m1o, po) in blk["poss"]:
                            w_sb = w_load(p)
                            ho = a0 + hrel
                            rel = boff + (m0o - gm0)
                            nc.tensor.matmul(
                                pio[:, rel:rel + M0],
                                w_sb[:, (j + 1) * H:(j + 2) * H],
                                ain[:, ho:ho + M0],
                                start=True, stop=(w0 == 0))
                            if w0:
                                nc.tensor.matmul(
                                    pio[:, rel + M0 - w0:rel + M0],
                                    w_sb[:, (j + 1) * H:(j + 2) * H],
                                    ain[:, a0 + e0:a0 + e0 + w0],
                                    start=False, stop=True)
                    nc.scalar.activation(si_all[:, gm0:gm0 + M0g],
                                         pio[:, 0:M0g], Sig,
                                         bias=bias_sb[:, 10:11])
                    nc.scalar.activation(so_all[:, gm0:gm0 + M0g],
                                         pio[:, 512:512 + M0g], Sig,
                                         bias=bias_sb[:, 11:12])
                    nc.scalar.activation(tu_all[:, gm0:gm0 + M0g],
                                         pio[:, 1024:1024 + M0g], Tanh,
                                         bias=bias_sb[:, 12:13])

                    # c_red = f * child_c (mask0 -> scratch, mask1 -> OUT)
                    for (p, C, M0, M1, hrel, e0, w0, e1, w1, m0o,
                         m1o, po) in blk["poss"]:
                        co = a0 + Cg + hrel
                        fo = po
                        nc.vector.tensor_mul(cr0[:, m0o:m0o + M0],
                                             f_all[:, fo:fo + M0],
                                             ain[:, co:co + M0])
                        if M1:
                            nc.vector.tensor_mul(
                                out[:, 2 * M0t + m1o:2 * M0t + m1o + M1],
                                f_all[:, fo + M0:fo + C],
                                ain[:, co + M0:co + C])

                # c_red(mask1) region complete
                nc.sync.dma_start(out=OUT[:, 2 * M0t:], in_=out[:, 2 * M0t:])

                # node func over body-wide 1024-col spans; tanh(c_new) is
                # one 2048-col ACT instruction (SBUF source, no bank limit)
                c_new = out[:, M0t:2 * M0t]
                nsp = (M0t + 1023) // 1024
                for s in range(nsp):
                    lo = s * 1024
                    hi = min(lo + 1024, M0t)
                    nc.vector.tensor_mul(prod[:, lo:hi], si_all[:, lo:hi],
                                         tu_all[:, lo:hi])
                    nc.vector.tensor_add(c_new[:, lo:hi], prod[:, lo:hi],
                                         cr0[:, lo:hi])
                nc.scalar.activation(th[:, :], c_new[:, :], Tanh)
                for s in range(nsp):
                    lo = s * 1024
                    hi = min(lo + 1024, M0t)
                    nc.vector.tensor_mul(out[:, lo:hi], so_all[:, lo:hi],
                                         th[:, lo:hi])
                nc.sync.dma_start(out=OUT[:, M0t:2 * M0t], in_=c_new[:, :])
                nc.sync.dma_start(out=OUT[:, 0:M0t], in_=out[:, 0:M0t])

            if reps == 1:
                for _ in range(bodies):
                    body()
            else:
                for p_ in range(N_POS):
                    w_load(p_)
                base, rem = divmod(reps, FUSE)
                if base:
                    with tc.For_i(0, base, 1) as _i:
                        for _ in range(FUSE * bodies):
                            body(_i)
                for _ in range(rem * bodies):
                    body()
    nc.finalize()
    return nc


_BUILD_CACHE = {}


def _prepare(inputs, reps=1, bodies=1):
    global N, H, N_POS, SH
    N, _, H = np.asarray(inputs["child_h"]).shape
    N_POS = np.asarray(inputs["W_f"]).shape[0] // H
    SH = N // NC
    child_h = np.asarray(inputs["child_h"], np.float32).reshape(N, H)
    child_c = np.asarray(inputs["child_c"], np.float32).reshape(N, H)
    e1 = np.asarray(inputs["extra_input_depth_1"], np.float32)
    e2 = np.asarray(inputs["extra_input_depth_2"], np.float32)
    h_prev = np.asarray(inputs["h_prev"], np.float32)
    pos = np.asarray(inputs["pos"]).astype(np.int64)
    depth = np.asarray(inputs["depth"]).astype(np.int64)
    mask = np.asarray(inputs["mask"]).astype(np.int64)
    W_f = np.asarray(inputs["W_f"], np.float32)
    b_f = np.asarray(inputs["b_f"], np.float32)
    W_iou = np.asarray(inputs["W_iou"], np.float32)
    b_iou = np.asarray(inputs["b_iou"], np.float32)

    mask01 = (mask != 0).astype(np.int64)
    plan = _plan(pos, depth, mask01)

    key = (tuple(tuple(g["poss"]) for g in plan["groups"]),
           plan["Lin"], plan["Lout"], reps, bodies)
    if key not in _BUILD_CACHE:
        _BUILD_CACHE[key] = _build(plan, reps=reps, bodies=bodies)
    nc = _BUILD_CACHE[key]

    # weights packed [H, 10*4*H]: per pos p: [W_f_p | Wi^T | Wo^T | Wu^T]
    Wp = np.empty((H, N_POS * 4 * H), np.float32)
    W_f_r = W_f.reshape(N_POS, H, H)
    for p in range(N_POS):
        base = p * 4 * H
        Wp[:, base:base + H] = W_f_r[p]
        for j in range(3):
            Wp[:, base + (j + 1) * H:base + (j + 2) * H] = \
                W_iou[j * H:(j + 1) * H, p * H:(p + 1) * H].T
    bias = np.empty((H, 13), np.float32)
    bias[:, :N_POS] = b_f.reshape(N_POS, H).T
    bias[:, 10] = b_iou[0, 0:H]
    bias[:, 11] = b_iou[0, H:2 * H]
    bias[:, 12] = b_iou[0, 2 * H:3 * H]

    # e source per node: e1 where depth==1, e2 where depth==2 (others unused)
    e_src = np.where((depth == 1)[:, None], e1, e2).astype(np.float32)
    srcs = (child_h, child_c, e_src)

    slot_idx = plan["slot_idx"]
    ain_slot, ain_kind = plan["ain_slot"], plan["ain_kind"]
    out_slot, out_kind = plan["out_slot"], plan["out_kind"]
    Lin = plan["Lin"]

    Wp16 = Wp.astype(BF)
    in_maps = []
    for c in range(NC):
        node = slot_idx[c][ain_slot]          # [Lin] node per ain col, -1 pad
        AIN = np.zeros((H, Lin), BF)
        for kind in range(3):
            m = (ain_kind == kind) & (node >= 0)
            AIN[:, m] = srcs[kind][node[m]].T.astype(BF)
        in_maps.append({"AIN": AIN, "W": Wp16, "BIAS": bias})

    mask_on = mask != 0

    def assemble(results):
        h = np.empty((N, H), np.float32)
        cc = np.empty((N, H), np.float32)
        for c in range(NC):
            node = slot_idx[c][out_slot]      # [Lout] node per out col
            O = np.asarray(results[c]["OUT"]).astype(np.float32)
            mh = (out_kind == 0) & (node >= 0)
            h[node[mh]] = O[:, mh].T
            mc = (out_kind != 0) & (node >= 0)
            cc[node[mc]] = O[:, mc].T
        h[mask_on] = h_prev[mask_on]
        return h, cc

    return nc, in_maps, assemble


def kernel(**inputs):
    nc, in_maps, assemble = _prepare(inputs)
    try:
        res = run_bass_kernel_spmd(nc, in_maps, list(range(NC)))
    except Exception:
        # first execution of a freshly compiled NEFF occasionally kills the
        # worker (transient); one retry has always succeeded
        res = run_bass_kernel_spmd(nc, in_maps, list(range(NC)))
    LAST["results"] = res
    LAST["nc"] = nc
    return assemble(res.results)


# revision 11
# speedup vs baseline: 1.1114x; 1.1114x over previous
"""DecoderTreeLSTMCell Trainium2 Bass kernel (v4).

Strategy: data-parallel over nodes on 8 cores (4096 nodes/core). Host-side,
each core's nodes are sorted by `pos` (10 blocks); within each pos block
[mask=0 | mask=1], each side sub-ordered [depth!=1,2 | d==1 | d==2] with
padded compile-time capacities. All device I/O is bf16 (rel-err gate is 2e-2;
bf16 lands ~6e-3), which halves HBM traffic vs fp32 AND avoids the fp32r
<256-col 4x PE streaming penalty.

AIN is laid out in 5 group blocks (2 positions each): [h | c | e] per group,
DMA'd in 3 slabs (~0.9MB each) so compute starts ~1.5us into a pass while
later slabs stream. OUT has global regions [h_new | c_new | c_red_mask1]
written by 3 DMAs as each region completes.

The depth-1/2 "extra input" adds are folded into the matmuls: gate matmuls
accumulate W^T @ e over the depth-1,2 column sub-ranges directly in PSUM
(same stationary weights, start=False) - exact same math as adding e to
h_cat, minus 20 vector ops. The i/o/u-gate matmuls of a 2-pos group share
PSUM banks ([i_p0|i_p1]...) so each sigmoid/tanh is ONE ACT instruction per
group (ACT's ~190ns/instr bubble is the #1 cost). Forget-gate f is per-pos
(bias b_f[pos] applied free by ACT). Elementwise work runs on DVE in bf16
over 1024-col spans of body-wide tiles.

The timing loop fuses 4 harness iterations per For_i trip (8 passes/trip):
the For_i back-edge costs an all-engine barrier + pipeline refill, a harness
artifact that would otherwise be charged to every 2nd pass. Total pass count
is exactly reps*bodies (remainder passes are emitted outside the loop).
mask=1 rows of h equal h_prev exactly and are filled host-side during
unshard (data routing only - all arithmetic happens on device).
"""
import numpy as np
import ml_dtypes

import concourse.bacc as bacc
import concourse.mybir as mybir
from concourse.tile import TileContext
from concourse.bass_utils import run_bass_kernel_spmd

N = 32768
H = 128
N_POS = 10
NC = 8
SH = N // NC  # nodes per core

F32 = mybir.dt.float32
BF16 = mybir.dt.bfloat16
BF = ml_dtypes.bfloat16
Sig = mybir.ActivationFunctionType.Sigmoid
Tanh = mybir.ActivationFunctionType.Tanh

FUSE = 8          # harness iterations fused per For_i trip
IN_SLABS = ((0, 2), (2, 4), (4, 5))   # group ranges per input DMA

# module-level stash for test harness introspection
LAST = {}


def _roundup(x, m):
    return ((x + m - 1) // m) * m


def _plan(pos, depth, mask):
    """Per-core slot layout: pos-major blocks [m0:k0,k1,k2 | m1:k0,k1,k2].

    AIN: 5 group blocks (2 pos each) of [h | c | e]. OUT: global regions
    [h_new (M0t) | c_new (M0t) | c_red_mask1 (M1t)].
    """
    dcl = np.where(depth == 1, 1, np.where(depth == 2, 2, 0))
    idx = {}
    counts = np.zeros((NC, N_POS, 2, 3), np.int64)
    # deal each (pos, mask, dclass) bucket round-robin across cores so
    # per-core counts are equal +-1 -> capacities carry almost no padding
    for p in range(N_POS):
        for m in range(2):
            for k in range(3):
                gg = np.nonzero((pos == p) & (mask == m) & (dcl == k))[0]
                for c in range(NC):
                    ii = gg[c::NC]
                    idx[(c, p, m, k)] = ii
                    counts[c, p, m, k] = len(ii)

    caps = np.zeros((N_POS, 2, 3), np.int64)
    for p in range(N_POS):
        for m in range(2):
            for k in range(3):
                caps[p, m, k] = _roundup(int(counts[:, p, m, k].max()), 4)

    M0 = caps[:, 0, :].sum(axis=1)       # [N_POS]
    M1 = caps[:, 1, :].sum(axis=1)
    C = M0 + M1
    P_off = np.concatenate([[0], np.cumsum(C)])       # slot offset per pos
    M0_off = np.concatenate([[0], np.cumsum(M0)])
    M1_off = np.concatenate([[0], np.cumsum(M1)])
    L = int(P_off[-1])
    M0t, M1t = int(M0.sum()), int(M1.sum())
    assert C.max() <= 512, f"pos block too wide for a PSUM bank: {C.max()}"

    slot_idx = np.full((NC, L), -1, np.int64)
    for c in range(NC):
        for p in range(N_POS):
            for m in range(2):
                o = int(P_off[p] + (0 if m == 0 else M0[p]))
                for k in range(3):
                    ii = idx[(c, p, m, k)]
                    slot_idx[c, o:o + len(ii)] = ii
                    o += int(caps[p, m, k])

    groups = []
    ain_slot, ain_kind = [], []
    a = 0
    for g in range(N_POS // 2):
        p0, p1 = 2 * g, 2 * g + 1
        Cg = int(C[p0] + C[p1])
        M0g = int(M0[p0] + M0[p1])
        assert M0g <= 512
        poss = []
        hrel = 0
        eoff = 2 * Cg
        for p in (p0, p1):
            w0 = int(caps[p, 0, 1] + caps[p, 0, 2])
            w1 = int(caps[p, 1, 1] + caps[p, 1, 2])
            poss.append((p, int(C[p]), int(M0[p]), int(M1[p]), hrel,
                         eoff, w0, eoff + w0, w1,
                         int(M0_off[p]), int(M1_off[p]), int(P_off[p])))
            eoff += w0 + w1
            hrel += int(C[p])
        Ag = eoff
        groups.append(dict(a0=a, Ag=Ag, Cg=Cg, M0g=M0g,
                           gm0=int(M0_off[p0]), gi=g * 1024, poss=poss))
        # AIN columns for this group: h | c | e
        for p in (p0, p1):
            ain_slot.extend(range(int(P_off[p]), int(P_off[p] + C[p])))
            ain_kind.extend([0] * int(C[p]))
        for p in (p0, p1):
            ain_slot.extend(range(int(P_off[p]), int(P_off[p] + C[p])))
            ain_kind.extend([1] * int(C[p]))
        for p in (p0, p1):
            w0 = int(caps[p, 0, 1] + caps[p, 0, 2])
            ain_slot.extend(range(int(P_off[p] + M0[p] - w0),
                                  int(P_off[p] + M0[p])))
            ain_kind.extend([2] * w0)
            w1 = int(caps[p, 1, 1] + caps[p, 1, 2])
            ain_slot.extend(range(int(P_off[p] + C[p] - w1),
                                  int(P_off[p] + C[p])))
            ain_kind.extend([2] * w1)
        a += Ag
    Lin = a

    out_slot, out_kind = [], []
    for p in range(N_POS):
        out_slot.extend(range(int(P_off[p]), int(P_off[p] + M0[p])))
        out_kind.extend([0] * int(M0[p]))
    for p in range(N_POS):
        out_slot.extend(range(int(P_off[p]), int(P_off[p] + M0[p])))
        out_kind.extend([1] * int(M0[p]))
    for p in range(N_POS):
        out_slot.extend(range(int(P_off[p] + M0[p]), int(P_off[p] + C[p])))
        out_kind.extend([2] * int(M1[p]))
    Lout = 2 * M0t + M1t

    return dict(groups=groups, L=L, Lin=Lin, Lout=Lout, M0t=M0t, M1t=M1t,
                slot_idx=slot_idx,
                ain_slot=np.array(ain_slot), ain_kind=np.array(ain_kind),
                out_slot=np.array(out_slot), out_kind=np.array(out_kind))


def _build(plan, reps=1, bodies=1):
    groups = plan["groups"]
    Lin, Lout = plan["Lin"], plan["Lout"]
    M0t, M1t = plan["M0t"], plan["M1t"]
    nc = bacc.Bacc("TRN2", target_bir_lowering=False)
    AIN = nc.dram_tensor("AIN", [H, Lin], BF16, kind="ExternalInput")
    W = nc.dram_tensor("W", [H, N_POS * 4 * H], BF16, kind="ExternalInput")
    BIAS = nc.dram_tensor("BIAS", [H, 13], F32, kind="ExternalInput")
    BT2 = nc.dram_tensor("BT2", [2, H], BF16, kind="ExternalInput")
    IND = nc.dram_tensor("IND", [2, 5 * 1024], BF16, kind="ExternalInput")
    OUT = nc.dram_tensor("OUT", [H, Lout], BF16, kind="ExternalOutput")

    with TileContext(nc) as tc:
        with (
            tc.tile_pool(name="const", bufs=1) as cpool,
            tc.tile_pool(name="io", bufs=2) as io,
            tc.tile_pool(name="gates", bufs=2) as gates,
            tc.tile_pool(name="wk", bufs=2) as wk,
            tc.tile_pool(name="ps_u", bufs=2, space="PSUM") as ps_u,
            tc.tile_pool(name="ps_io", bufs=2, space="PSUM") as ps_io,
            tc.tile_pool(name="ps_t", bufs=2, space="PSUM") as ps_t,
        ):
            bias_sb = cpool.tile([H, 13], F32, tag="bias")
            nc.sync.dma_start(out=bias_sb[:, :], in_=BIAS[:, :])
            bt2_sb = cpool.tile([2, H], BF16, tag="bt2")
            nc.sync.dma_start(out=bt2_sb[:, :], in_=BT2[:, :])
            ind_sb = cpool.tile([2, 5 * 1024], BF16, tag="ind")
            nc.sync.dma_start(out=ind_sb[:, :], in_=IND[:, :])
            w_tiles = {}

            def w_load(p):
                if p not in w_tiles:
                    t = cpool.tile([H, 4 * H], BF16, tag=f"w{p}")
                    nc.sync.dma_start(
                        out=t[:, :], in_=W[:, p * 4 * H:(p + 1) * 4 * H])
                    w_tiles[p] = t
                return w_tiles[p]

            def body(_iv=None):
                ain = io.tile([H, Lin], BF16, tag="ain")
                for (glo, ghi) in IN_SLABS:
                    s0 = groups[glo]["a0"]
                    s1 = (groups[ghi]["a0"] if ghi < len(groups) else Lin)
                    nc.sync.dma_start(out=ain[:, s0:s1], in_=AIN[:, s0:s1])
                out = io.tile([H, Lout], BF16, tag="out")
                f_all = gates.tile([H, plan["L"]], BF16, tag="f")
                sio_all = gates.tile([H, 2 * M0t], BF16, tag="sio")
                tu_all = gates.tile([H, M0t], BF16, tag="tu")
                cr0 = wk.tile([H, M0t], BF16, tag="cr0")
                prod = wk.tile([H, M0t], BF16, tag="pr")
                th = wk.tile([H, M0t], BF16, tag="th")

                for blk in groups:
                    a0, Cg, M0g, gm0 = (blk["a0"], blk["Cg"], blk["M0g"],
                                        blk["gm0"])
                    # u matmuls + forget gate, per pos (e adds folded into
                    # PSUM accumulation; u split at the mask0/1 boundary so
                    # each e block is a suffix of its segment)
                    for (p, C, M0, M1, hrel, e0, w0, e1, w1, m0o,
                         m1o, po) in blk["poss"]:
                        w_sb = w_load(p)
                        ho = a0 + hrel
                        pu = ps_u.tile([H, C], F32, tag="u")
                        nc.tensor.matmul(pu[:, 0:M0], w_sb[:, 0:H],
                                         ain[:, ho:ho + M0],
                                         start=True, stop=(w0 == 0))
                        if w0:
                            nc.tensor.matmul(pu[:, M0 - w0:M0], w_sb[:, 0:H],
                                             ain[:, a0 + e0:a0 + e0 + w0],
                                             start=False, stop=True)
                        nc.tensor.matmul(pu[:, M0:C], w_sb[:, 0:H],
                                         ain[:, ho + M0:ho + C],
                                         start=True, stop=(w1 == 0))
                        if w1:
                            nc.tensor.matmul(pu[:, C - w1:C], w_sb[:, 0:H],
                                             ain[:, a0 + e1:a0 + e1 + w1],
                                             start=False, stop=True)
                        nc.scalar.activation(f_all[:, po:po + C],
                                             pu[:, :], Sig,
                                             bias=bias_sb[:, p:p + 1])

                    # i/o gates packed [i_p0|i_p1|o_p0|o_p1] over 2 PSUM
                    # banks. Each bank is one accumulation group: a K=2 bias
                    # matmul (rows b_i/b_o x indicator) opens it (start=True)
                    # over the FULL bank, gate matmuls accumulate, the last
                    # one closes it (stop=True). -> ONE sigmoid, no pads.
                    gi = blk["gi"]
                    pio = ps_io.tile([H, 1024], F32, tag="io")
                    mms = []   # (lo, hi, w_col, src_off) pending accumulates
                    for (p, C, M0, M1, hrel, e0, w0, e1, w1, m0o,
                         m1o, po) in blk["poss"]:
                        ho = a0 + hrel
                        rel = m0o - gm0
                        mms.append((rel, rel + M0, p, 1, ho))
                        if w0:
                            mms.append((rel + M0 - w0, rel + M0, p, 1,
                                        a0 + e0))
                        mms.append((M0g + rel, M0g + rel + M0, p, 2, ho))
                        if w0:
                            mms.append((M0g + rel + M0 - w0, M0g + rel + M0,
                                        p, 2, a0 + e0))
                    pieces = []
                    for (lo, hi, p, j, so_) in mms:
                        x = lo
                        while x < hi:
                            y = min((x // 512 + 1) * 512, hi)
                            pieces.append((x, y, p, j, so_ + (x - lo)))
                            x = y
                    last_in_bank = {}
                    for n_, (x, y, p, j, so_) in enumerate(pieces):
                        last_in_bank[x // 512] = n_
                    for b in range(2):
                        nc.tensor.matmul(pio[:, 512 * b:512 * (b + 1)],
                                         bt2_sb[:, :],
                                         ind_sb[:, gi + 512 * b:
                                                gi + 512 * (b + 1)],
                                         start=True,
                                         stop=(b not in last_in_bank))
                    for n_, (x, y, p, j, so_) in enumerate(pieces):
                        w_sb = w_load(p)
                        nc.tensor.matmul(pio[:, x:y],
                                         w_sb[:, j * H:(j + 1) * H],
                                         ain[:, so_:so_ + (y - x)],
                                         start=False,
                                         stop=(last_in_bank[x // 512] == n_))
                    nc.scalar.activation(sio_all[:, 2 * gm0:
                                                 2 * gm0 + 2 * M0g],
                                         pio[:, 0:2 * M0g], Sig)
                    # u-gate (tanh) bank: per-pos groups, bias via ACT
                    pt = ps_t.tile([H, 512], F32, tag="t")
                    for (p, C, M0, M1, hrel, e0, w0, e1, w1, m0o,
                         m1o, po) in blk["poss"]:
                        w_sb = w_load(p)
                        ho = a0 + hrel
                        rel = m0o - gm0
                        nc.tensor.matmul(pt[:, rel:rel + M0],
                                         w_sb[:, 3 * H:4 * H],
                                         ain[:, ho:ho + M0],
                                         start=True, stop=(w0 == 0))
                        if w0:
                            nc.tensor.matmul(pt[:, rel + M0 - w0:rel + M0],
                                             w_sb[:, 3 * H:4 * H],
                                             ain[:, a0 + e0:a0 + e0 + w0],
                                             start=False, stop=True)
                    nc.scalar.activation(tu_all[:, gm0:gm0 + M0g],
                                         pt[:, 0:M0g], Tanh,
                                         bias=bias_sb[:, 12:13])

                    # c_red = f * child_c (mask0 -> scratch, mask1 -> OUT)
                    for (p, C, M0, M1, hrel, e0, w0, e1, w1, m0o,
                         m1o, po) in blk["poss"]:
                        co = a0 + Cg + hrel
                        fo = po
                        nc.vector.tensor_mul(cr0[:, m0o:m0o + M0],
                                             f_all[:, fo:fo + M0],
                                             ain[:, co:co + M0])
                        if M1:
                            nc.vector.tensor_mul(
                                out[:, 2 * M0t + m1o:2 * M0t + m1o + M1],
                                f_all[:, fo + M0:fo + C],
                                ain[:, co + M0:co + C])

                # c_red(mask1) region complete
                nc.sync.dma_start(out=OUT[:, 2 * M0t:], in_=out[:, 2 * M0t:])

                # node func: prod/h per group (sio slices), c_new in 1024
                # spans, tanh(c_new) as one 2048-col ACT instruction
                c_new = out[:, M0t:2 * M0t]
                for blk in groups:
                    gm0, M0g = blk["gm0"], blk["M0g"]
                    nc.vector.tensor_mul(prod[:, gm0:gm0 + M0g],
                                         sio_all[:, 2 * gm0:2 * gm0 + M0g],
                                         tu_all[:, gm0:gm0 + M0g])
                nsp = (M0t + 1023) // 1024
                for s in range(nsp):
                    lo = s * 1024
                    hi = min(lo + 1024, M0t)
                    nc.vector.tensor_add(c_new[:, lo:hi], prod[:, lo:hi],
                                         cr0[:, lo:hi])
                nc.scalar.activation(th[:, :], c_new[:, :], Tanh)
                for blk in groups:
                    gm0, M0g = blk["gm0"], blk["M0g"]
                    nc.vector.tensor_mul(
                        out[:, gm0:gm0 + M0g],
                        sio_all[:, 2 * gm0 + M0g:2 * gm0 + 2 * M0g],
                        th[:, gm0:gm0 + M0g])
                nc.sync.dma_start(out=OUT[:, M0t:2 * M0t], in_=c_new[:, :])
                nc.sync.dma_start(out=OUT[:, 0:M0t], in_=out[:, 0:M0t])

            if reps == 1:
                for _ in range(bodies):
                    body()
            else:
                for p_ in range(N_POS):
                    w_load(p_)
                base, rem = divmod(reps, FUSE)
                if base:
                    with tc.For_i(0, base, 1) as _i:
                        for _ in range(FUSE * bodies):
                            body(_i)
                for _ in range(rem * bodies):
                    body()
    nc.finalize()
    return nc


_BUILD_CACHE = {}


def _prepare(inputs, reps=1, bodies=1):
    global N, H, N_POS, SH
    N, _, H = np.asarray(inputs["child_h"]).shape
    N_POS = np.asarray(inputs["W_f"]).shape[0] // H
    SH = N // NC
    child_h = np.asarray(inputs["child_h"], np.float32).reshape(N, H)
    child_c = np.asarray(inputs["child_c"], np.float32).reshape(N, H)
    e1 = np.asarray(inputs["extra_input_depth_1"], np.float32)
    e2 = np.asarray(inputs["extra_input_depth_2"], np.float32)
    h_prev = np.asarray(inputs["h_prev"], np.float32)
    pos = np.asarray(inputs["pos"]).astype(np.int64)
    depth = np.asarray(inputs["depth"]).astype(np.int64)
    mask = np.asarray(inputs["mask"]).astype(np.int64)
    W_f = np.asarray(inputs["W_f"], np.float32)
    b_f = np.asarray(inputs["b_f"], np.float32)
    W_iou = np.asarray(inputs["W_iou"], np.float32)
    b_iou = np.asarray(inputs["b_iou"], np.float32)

    mask01 = (mask != 0).astype(np.int64)
    plan = _plan(pos, depth, mask01)

    key = (tuple(tuple(g["poss"]) for g in plan["groups"]),
           plan["Lin"], plan["Lout"], reps, bodies)
    if key not in _BUILD_CACHE:
        _BUILD_CACHE[key] = _build(plan, reps=reps, bodies=bodies)
    nc = _BUILD_CACHE[key]

    # weights packed [H, 10*4*H]: per pos p: [W_f_p | Wi^T | Wo^T | Wu^T]
    Wp = np.empty((H, N_POS * 4 * H), np.float32)
    W_f_r = W_f.reshape(N_POS, H, H)
    for p in range(N_POS):
        base = p * 4 * H
        Wp[:, base:base + H] = W_f_r[p]
        for j in range(3):
            Wp[:, base + (j + 1) * H:base + (j + 2) * H] = \
                W_iou[j * H:(j + 1) * H, p * H:(p + 1) * H].T
    bias = np.empty((H, 13), np.float32)
    bias[:, :N_POS] = b_f.reshape(N_POS, H).T
    bias[:, 10] = b_iou[0, 0:H]
    bias[:, 11] = b_iou[0, H:2 * H]
    bias[:, 12] = b_iou[0, 2 * H:3 * H]
    bt2 = np.stack([b_iou[0, 0:H], b_iou[0, H:2 * H]]).astype(BF)
    ind = np.zeros((2, 5 * 1024), BF)
    for g_, blk_ in enumerate(plan["groups"]):
        M0g_ = blk_["M0g"]
        ind[0, g_ * 1024:g_ * 1024 + M0g_] = 1
        ind[1, g_ * 1024 + M0g_:(g_ + 1) * 1024] = 1

    # e source per node: e1 where depth==1, e2 where depth==2 (others unused)
    e_src = np.where((depth == 1)[:, None], e1, e2).astype(np.float32)
    srcs = (child_h, child_c, e_src)

    slot_idx = plan["slot_idx"]
    ain_slot, ain_kind = plan["ain_slot"], plan["ain_kind"]
    out_slot, out_kind = plan["out_slot"], plan["out_kind"]
    Lin = plan["Lin"]

    Wp16 = Wp.astype(BF)
    in_maps = []
    for c in range(NC):
        node = slot_idx[c][ain_slot]          # [Lin] node per ain col, -1 pad
        AIN = np.zeros((H, Lin), BF)
        for kind in range(3):
            m = (ain_kind == kind) & (node >= 0)
            AIN[:, m] = srcs[kind][node[m]].T.astype(BF)
        in_maps.append({"AIN": AIN, "W": Wp16, "BIAS": bias,
                        "BT2": bt2, "IND": ind})

    mask_on = mask != 0

    def assemble(results):
        h = np.empty((N, H), np.float32)
        cc = np.empty((N, H), np.float32)
        for c in range(NC):
            node = slot_idx[c][out_slot]      # [Lout] node per out col
            O = np.asarray(results[c]["OUT"]).astype(np.float32)
            mh = (out_kind == 0) & (node >= 0)
            h[node[mh]] = O[:, mh].T
            mc = (out_kind != 0) & (node >= 0)
            cc[node[mc]] = O[:, mc].T
        h[mask_on] = h_prev[mask_on]
        return h, cc

    return nc, in_maps, assemble


def kernel(**inputs):
    nc, in_maps, assemble = _prepare(inputs)
    try:
        res = run_bass_kernel_spmd(nc, in_maps, list(range(NC)))
    except Exception:
        # first execution of a freshly compiled NEFF occasionally kills the
        # worker (transient); one retry has always succeeded
        res = run_bass_kernel_spmd(nc, in_maps, list(range(NC)))
    LAST["results"] = res
    LAST["nc"] = nc
    return assemble(res.results)
